# revision 8
# baseline (speedup 1.0000x reference)
"""TRN2 Bass kernel: ClapAudio window self-attention (B=2048 windows of 64
tokens, C=256, 8 heads x d=32), data-parallel over windows across 8 cores.

Host side: shards hidden_states, precomputes EB = exp(rel-pos-bias + mask)^T
(folding both additive score biases into one multiplicative table applied
after exp), and passes transposed weights in bf16.

Device side (per core, 256 windows): see build() docstring.
"""

import numpy as np
import ml_dtypes

import concourse.bass as bass
import concourse.mybir as mybir
import concourse.tile as tile
from concourse.bass_utils import run_bass_kernel_spmd
from concourse.masks import make_identity

DT = mybir.dt
F32 = DT.float32
BF16 = DT.bfloat16

N_CORES = 8
B = 2048
C = 256
H = 8
D = 32
WINTOK = 64
SCALE = 1.0 / np.sqrt(np.float32(D))

def _wait_cap(inst):
    """Max sem waits the walrus encoding of this instruction tolerates."""
    if isinstance(inst, (mybir.InstDrain, mybir.InstNoOp)):
        return 1  # CTRL_NO_STRUCT
    if isinstance(inst, (mybir.InstDMACopy, mybir.InstDMA, mybir.InstDmaTransposeAnt)):
        return 1  # PSEUDO_DMA_DIRECT2D
    return 1


def split_drain_waits(nc):
    """Walrus instruction encodings only fit a limited number of sem waits;
    Tile can attach more. Hoist excess waits onto NoOps inserted before the
    instruction on the same engine (in-order sequencers make this
    equivalent, if slightly more conservative)."""
    for f in nc.m.functions:
        for bb in f.blocks:
            new_insts = []
            for inst in bb.instructions:
                si = inst.sync_info
                cap = _wait_cap(inst)
                if si is not None and si.on_wait and len(si.on_wait) > cap:
                    waits = list(si.on_wait)
                    keep, rest = waits[:cap], waits[cap:]
                    for i in range(0, len(rest), 1):
                        new_insts.append(
                            mybir.InstNoOp(
                                name=f"{inst.name}-waitsplit-{i}",
                                engine=inst.engine,
                                sync_info=mybir.SyncInfo(
                                    on_wait=[rest[i]], on_update=[]
                                ),
                            )
                        )
                    inst.sync_info = mybir.SyncInfo(
                        on_wait=keep, on_update=list(si.on_update or [])
                    )
                new_insts.append(inst)
            bb.instructions[:] = new_insts


def build(n_windows=256, chunk_windows=8, split_waits=True):
    """Emit the per-core kernel.

    Layouts:
      x    DRAM [ntok, 256] bf16
      eb   DRAM [32, 128, 512] bf16:
           eb[t, slot*64+k, 64h+q] = exp(rpb[h,q,k] + mask[2t+slot,q,k])
      wqt/wkt/wvt DRAM [256, 256] bf16 = W.T  ([C_in, C_out])
      bqv/bkv DRAM [256] f32 ; bvr DRAM [1, 256] bf16
      out  DRAM [ntok, 256] f32

    Per 512-token chunk (8 windows = 4 window pairs): DMA X in; X^T via PE
    transpose; weight-stationary Q^T/K^T projections (+bias on copy);
    X^T-stationary V projection (+bv via K=1 ones matmul) scattered into a
    block-diag augmented V (with ones cols for softmax sums); per
    window-head scores^T = K^T.T @ Q^T (K=d=32, tile_position round-robin);
    ACT exp (scale folded, no max-subtraction needed at these magnitudes);
    GPSIMD multiply by resident EB table; per-head pair-matmul
    probs.T @ V_aug giving unnormalized ctx + softmax sums; DVE reciprocal +
    broadcast-AP multiply to normalize straight into the f32 staging tile;
    2 DMAs out per window pair.
    """
    assert n_windows % chunk_windows == 0 and chunk_windows % 2 == 0
    ntok = n_windows * WINTOK
    n_chunks = n_windows // chunk_windows
    chunk_tok = chunk_windows * WINTOK  # 512
    n_tile = chunk_tok // 128  # 4 token-tiles (each = 1 window pair)
    assert chunk_tok == 512

    nc = bass.Bass()
    x = nc.declare_dram_parameter("x", [ntok, C], BF16, isOutput=False)
    eb = nc.declare_dram_parameter("eb", [32, 128, 512], BF16, isOutput=False)
    wqt = nc.declare_dram_parameter("wqt", [C, C], BF16, isOutput=False)
    wkt = nc.declare_dram_parameter("wkt", [C, C], BF16, isOutput=False)
    wvt = nc.declare_dram_parameter("wvt", [C, C], BF16, isOutput=False)
    bqv = nc.declare_dram_parameter("bqv", [C], F32, isOutput=False)
    bkv = nc.declare_dram_parameter("bkv", [C], F32, isOutput=False)
    bvr = nc.declare_dram_parameter("bvr", [128, C], BF16, isOutput=False)
    out = nc.declare_dram_parameter("out", [ntok, C], F32, isOutput=True)

    with tile.TileContext(nc) as tc:
        with (
            tc.tile_pool(name="const", bufs=1) as cpool,
            tc.tile_pool(name="xin", bufs=2) as xpool,
            tc.tile_pool(name="acts", bufs=2) as apool,
            tc.tile_pool(name="probs", bufs=3) as ppool,
            tc.tile_pool(name="stage", bufs=3) as spool,
            tc.tile_pool(name="small", bufs=3) as smpool,
            tc.tile_pool(name="vv", bufs=3) as vpool,
            tc.tile_pool(name="pp", bufs=3, space="PSUM") as pp,
            tc.tile_pool(name="psc", bufs=3, space="PSUM") as psc,
            tc.tile_pool(name="pctx", bufs=2, space="PSUM") as pctx,
        ):
            # ---- constants ----
            eb_sb = cpool.tile([128, 32 * 512], BF16)
            for t in range(32):
                nc.sync.dma_start(eb_sb[:, t * 512 : (t + 1) * 512], eb[t, :, :])
            wq_sb = cpool.tile([128, 512], BF16)
            wk_sb = cpool.tile([128, 512], BF16)
            wv_sb = cpool.tile([128, 512], BF16)
            for w_sb, w_dram in ((wq_sb, wqt), (wk_sb, wkt), (wv_sb, wvt)):
                for ck in range(2):
                    nc.sync.dma_start(
                        w_sb[:, ck * 256 : (ck + 1) * 256],
                        w_dram[ck * 128 : (ck + 1) * 128, :],
                    )
            bq_sb = cpool.tile([128, 2], F32)
            bk_sb = cpool.tile([128, 2], F32)
            nc.sync.dma_start(bq_sb[:], bqv.rearrange("(m p) -> p m", p=128))
            nc.sync.dma_start(bk_sb[:], bkv.rearrange("(m p) -> p m", p=128))
            # bv as K=128 constant matmul: lhsT = 1/128, rhs = bv broadcast
            inv_sb = cpool.tile([128, 128], BF16)
            nc.vector.memset(inv_sb[:], 1.0 / 128.0)
            bvb_sb = cpool.tile([128, C], BF16)
            nc.sync.dma_start(bvb_sb[:], bvr[:, :])
            ident = cpool.tile([128, 128], BF16)
            make_identity(nc, ident[:])

            # static vaugs: [128, 264] per window, double-buffered (4 tiles).
            # rows 0:64 pair-block j cols 66j+dv = V_win[:, head 2j]; ones at
            # 66j+32. rows 64:128 cols 66j+33+dv = V_win[:, head 2j+1]; ones
            # at 66j+65.
            vaugs = []
            for vb in range(4):
                va = cpool.tile([128, 264], BF16, tag=f"vaug{vb}")
                nc.vector.memset(va[:], 0.0)
                nc.vector.memset(
                    va[0:64, :].rearrange("p (j c) -> p j c", j=4)[:, :, 32:33], 1.0
                )
                nc.vector.memset(
                    va[64:128, :].rearrange("p (j c) -> p j c", j=4)[:, :, 65:66], 1.0
                )
                vaugs.append(va)

            # ---- main loop over chunks ----
            for u in range(n_chunks):
                t0 = u * chunk_tok

                x_sb = xpool.tile([128, n_tile * C], BF16, tag="x")
                nc.sync.dma_start(
                    x_sb.rearrange("p (i c) -> p i c", i=n_tile),
                    x[t0 : t0 + chunk_tok, :].rearrange("(i p) c -> p i c", p=128),
                )

                # X^T via PE transpose
                xt_sb = apool.tile([128, 2 * 512], BF16, tag="xt")
                for ck in range(2):
                    xtp = pp.tile([128, 512], BF16, tag="pp")
                    for i in range(n_tile):
                        nc.tensor.transpose(
                            xtp[:, i * 128 : (i + 1) * 128],
                            x_sb[:, i * C + ck * 128 : i * C + ck * 128 + 128],
                            ident[:],
                        )
                    cp_out = xt_sb[:, ck * 512 : (ck + 1) * 512]
                    if ck == 0:
                        nc.scalar.copy(cp_out, xtp[:])
                    else:
                        nc.vector.tensor_copy(cp_out, xtp[:])

                # Q^T / K^T projections (weights stationary)
                qt_sb = apool.tile([128, 2 * 512], BF16, tag="qt")
                kt_sb = apool.tile([128, 2 * 512], BF16, tag="kt")
                for qk, (w_sb, b_sb, dst) in enumerate(
                    ((wq_sb, bq_sb, qt_sb), (wk_sb, bk_sb, kt_sb))
                ):
                    for m in range(2):
                        prj = pp.tile([128, 512], F32, tag="pp")
                        for ck in range(2):
                            nc.tensor.matmul(
                                prj[:],
                                w_sb[:, ck * 256 + m * 128 : ck * 256 + (m + 1) * 128],
                                xt_sb[:, ck * 512 : (ck + 1) * 512],
                                start=(ck == 0),
                                stop=(ck == 1),
                            )
                        cp_out = dst[:, m * 512 : (m + 1) * 512]
                        if qk == 0:
                            nc.scalar.activation(
                                cp_out,
                                prj[:],
                                mybir.ActivationFunctionType.Identity,
                                bias=b_sb[:, m : m + 1],
                            )
                        else:
                            nc.vector.tensor_scalar_add(
                                cp_out, prj[:], b_sb[:, m : m + 1]
                            )

                # half-shift copies: heads h%4 in {2,3} live at rows 64:128;
                # move them to rows 0:64 so every K=32 matmul uses strips 0/32
                qt2_sb = apool.tile([64, 2 * 512], BF16, tag="qt2")
                kt2_sb = apool.tile([64, 2 * 512], BF16, tag="kt2")
                nc.sync.dma_start(qt2_sb[:], qt_sb[64:128, :])
                nc.sync.dma_start(kt2_sb[:], kt_sb[64:128, :])

                def qk_slice(src, src2, h, cols):
                    r = h % 4
                    m = h // 4
                    if r < 2:
                        return src[32 * r : 32 * r + 32, m * 512 + cols[0] : m * 512 + cols[1]]
                    return src2[32 * (r - 2) : 32 * (r - 2) + 32, m * 512 + cols[0] : m * 512 + cols[1]]

                # V projection + bv
                for i in range(n_tile):
                    vps = pp.tile([128, 256], F32, tag="pp")
                    for ck in range(2):
                        nc.tensor.matmul(
                            vps[:],
                            xt_sb[:, ck * 512 + i * 128 : ck * 512 + (i + 1) * 128],
                            wv_sb[:, ck * 256 : (ck + 1) * 256],
                            start=(ck == 0),
                            stop=False,
                        )
                    nc.tensor.matmul(
                        vps[:], inv_sb[:, :], bvb_sb[:, :], start=False, stop=True
                    )
                    vtmp = vpool.tile([128, 256], BF16, tag="vt")
                    nc.vector.tensor_copy(vtmp[:], vps[:])
                    vswap = vpool.tile([128, 256], BF16, tag="vs")
                    nc.sync.dma_start(vswap[64:128, :], vtmp[0:64, :])
                    nc.sync.dma_start(vswap[0:64, :], vtmp[64:128, :])
                    va_a = vaugs[2 * (i % 2)]
                    va_b = vaugs[2 * (i % 2) + 1]
                    # vaug_A: top = V(wA) even heads, bottom = V(wA) odd heads
                    nc.vector.tensor_copy(
                        va_a[0:64, :].rearrange("p (j c) -> p j c", j=4)[:, :, 0:32],
                        vtmp[0:64, :].rearrange("p (j c) -> p j c", j=4)[:, :, 0:32],
                    )
                    nc.scalar.copy(
                        va_a[64:128, :].rearrange("p (j c) -> p j c", j=4)[:, :, 33:65],
                        vswap[64:128, :].rearrange("p (j c) -> p j c", j=4)[:, :, 32:64],
                    )
                    # vaug_B: top = V(wB) even heads, bottom = V(wB) odd heads
                    nc.vector.tensor_copy(
                        va_b[0:64, :].rearrange("p (j c) -> p j c", j=4)[:, :, 0:32],
                        vswap[0:64, :].rearrange("p (j c) -> p j c", j=4)[:, :, 0:32],
                    )
                    nc.scalar.copy(
                        va_b[64:128, :].rearrange("p (j c) -> p j c", j=4)[:, :, 33:65],
                        vtmp[64:128, :].rearrange("p (j c) -> p j c", j=4)[:, :, 32:64],
                    )

                    # ---- attention for this window pair ----
                    wp = u * n_tile + i
                    nwp = wp % 32
                    scp = psc.tile([128, 512], F32, tag="sc")
                    for h in range(H):
                        s = 32 * (h % 2)
                        for win in range(2):
                            cols = (i * 128 + win * 64, i * 128 + win * 64 + 64)
                            b = (h // 2) * 2 + win
                            nc.tensor.matmul(
                                scp[s * 2 : s * 2 + 64, b * 64 : b * 64 + 64],
                                qk_slice(kt_sb, kt2_sb, h, cols),
                                qk_slice(qt_sb, qt2_sb, h, cols),
                                start=True,
                                stop=True,
                                tile_position=(s, s * 2),
                            )
                    probs = ppool.tile([128, 512], BF16, tag="pr")
                    nc.scalar.activation(
                        probs[:],
                        scp[:],
                        mybir.ActivationFunctionType.Exp,
                        scale=float(SCALE),
                    )
                    nc.gpsimd.tensor_mul(
                        probs[:], probs[:], eb_sb[:, nwp * 512 : (nwp + 1) * 512]
                    )

                    ctxp = pctx.tile([128, 264], F32, tag="ctx")
                    for j in range(4):
                        for win in range(2):
                            va = va_a if win == 0 else va_b
                            nc.tensor.matmul(
                                ctxp[win * 64 : win * 64 + 64, j * 66 : j * 66 + 66],
                                probs[:, (j * 2 + win) * 64 : (j * 2 + win) * 64 + 64],
                                va[:, j * 66 : j * 66 + 66],
                                start=True,
                                stop=True,
                                tile_position=(0, 64 * win),
                            )

                    recips = smpool.tile([128, 8], F32, tag="rc")
                    sums_ap = ctxp.rearrange("p (j par c) -> p j par c", j=4, par=2)[
                        :, :, :, 32:33
                    ]
                    nc.vector.reciprocal(recips[:], sums_ap)

                    stg = spool.tile([128, 256], F32, tag="st")
                    ctx_ap = ctxp.rearrange("p (j par c) -> p j par c", j=4, par=2)[
                        :, :, :, 0:32
                    ]
                    rec_ap = recips.rearrange("p (j par one) -> p j par one", j=4, one=1)
                    ctx_b, rec_b = bass.broadcast_tensor_aps(ctx_ap, rec_ap)
                    out_ap = stg.rearrange("p (j par c) -> p j par c", j=4, par=2)
                    nc.vector.tensor_tensor(out_ap, ctx_b, rec_b, mybir.AluOpType.mult)

                    nc.sync.dma_start(out[wp * 128 : wp * 128 + 128, :], stg[:])

    if split_waits:
        split_drain_waits(nc)
    return nc


def ref_shard(x, eb_full, wq, bq, wk, bk, wv, bv):
    """NumPy reference for one shard (dev-time check)."""
    ntok = x.shape[0]
    nwin = ntok // WINTOK
    q = x @ wq.T + bq
    k = x @ wk.T + bk
    v = x @ wv.T + bv
    outp = np.zeros((ntok, C), np.float32)
    for w in range(nwin):
        t = slice(w * WINTOK, (w + 1) * WINTOK)
        nwp, slot = (w // 2) % 32, w % 2
        for h in range(H):
            qh = q[t, h * D : (h + 1) * D]
            kh = k[t, h * D : (h + 1) * D]
            vh = v[t, h * D : (h + 1) * D]
            sT = (kh @ qh.T) * SCALE
            b = (h // 2) * 2 + slot
            ebm = eb_full[nwp, (h % 2) * 64 : (h % 2) * 64 + 64, b * 64 : b * 64 + 64]
            pu = np.exp(sT) * ebm
            ctx = pu.T @ vh
            s = pu.sum(axis=0)
            outp[t, h * D : (h + 1) * D] = ctx / s[:, None]
    return outp


_NC_CACHE = {}


def _get_nc():
    key = "main"
    if key not in _NC_CACHE:
        _NC_CACHE[key] = build(n_windows=B // N_CORES)
    return _NC_CACHE[key]


def _pack_eb(bias_table, rel_index, attention_mask):
    # rpb[h, q, k] = bias_table[rel_index[q, k], h]
    rpb = bias_table[rel_index.reshape(-1)].reshape(64, 64, H).transpose(2, 0, 1)
    e = np.exp(
        rpb[None].astype(np.float64) + attention_mask[:, None].astype(np.float64)
    ).astype(np.float32)
    # e [nw, h, q, k] -> eb[t, (h%2)*64 + k, ((h//2)*2 + slot)*64 + q]
    e2 = e.transpose(0, 1, 3, 2)  # [nw, h, k, q]
    e3 = e2.reshape(32, 2, 4, 2, 64, 64)  # [t, slot, j, par, k, q]
    e4 = e3.transpose(0, 3, 4, 2, 1, 5)  # [t, par, k, j, slot, q]
    return np.ascontiguousarray(e4.reshape(32, 128, 512))


def kernel(
    hidden_states,
    attention_mask,
    Wq,
    bq,
    Wk,
    bk,
    Wv,
    bv,
    bias_table,
    rel_index,
):
    bf = ml_dtypes.bfloat16
    nc = _get_nc()

    xs = np.ascontiguousarray(hidden_states.reshape(B * WINTOK, C)).astype(bf)
    eb = _pack_eb(
        np.asarray(bias_table, np.float32),
        np.asarray(rel_index),
        np.asarray(attention_mask, np.float32),
    ).astype(bf)
    common = {
        "eb": eb,
        "wqt": np.ascontiguousarray(Wq.T).astype(bf),
        "wkt": np.ascontiguousarray(Wk.T).astype(bf),
        "wvt": np.ascontiguousarray(Wv.T).astype(bf),
        "bqv": np.asarray(bq, np.float32),
        "bkv": np.asarray(bk, np.float32),
        "bvr": np.tile(np.asarray(bv, np.float32)[None, :], (128, 1)).astype(bf),
    }
    shard_tok = (B // N_CORES) * WINTOK
    in_maps = [
        {"x": xs[c * shard_tok : (c + 1) * shard_tok], **common}
        for c in range(N_CORES)
    ]
    res = run_bass_kernel_spmd(nc, in_maps, list(range(N_CORES)))
    outp = np.concatenate(
        [res.results[c]["out"] for c in range(N_CORES)], axis=0
    )
    return outp.reshape(B, WINTOK, C).astype(np.float32)
